# revision 71
# baseline (speedup 1.0000x reference)
"""TRN2 Bass kernel for nn_EdgeMLP: masked pairwise cosine similarity.

out[i, j] = [cls1_i == cls2_j] * cos(f(e1_i), f(e2_j)),  f = 2-layer MLP.

Strategy (8 cores, block-diagonal over the class mask):
  - The mask [cls1_i == cls2_j] with 8 classes means only ~1/8 of the
    8192x8192 output is nonzero.  Sort edges1 rows AND edges2 columns by
    class (host-side, pure data movement): the nonzero support becomes 8
    dense class blocks [m_c, n_c].  Core c computes block c = the full
    cosine matrix between class-c rows and class-c columns -- no masking
    logic on device at all.  The host scatters the 8 blocks into the
    zero-initialized full output (the gather/unshard step).
  - All matmuls are f32r (tf32-like, 1 cyc/row): MLP layers, column
    squared-norms (ones-matmul), and the main [32]x[128,N] dot products.
  - Side-1 (rows) norms are computed PARTITION-major via tiny per-m-tile
    matmuls against a ones column-pair, so the 1/|f1_i| row scaling rides
    the PSUM->SBUF output copy for free (per-partition `scale` operand of
    the copy).  Side-2 norms use the free-major ones-matmul + sqrt(ACT) +
    reciprocal(DVE) + fused (f2+b2)*rn2 stt (DVE).
  - Output tiles are bf16 (rel err ~2e-3 << 2e-2 gate), halving out-DMA
    bytes; the host upcasts to f32 during the scatter.
  - PSUM is managed as 7 rotating 1-bank tiles (+1 for the long-lived
    norm accumulator); every matmul output fits one bank.  Prologue is
    pipelined in 512-column chunks; emission is choreographed so each
    engine's in-order queue receives ops in readiness order, with the
    first main matmuls and an m-tile-0 norm fast path emitted
    mid-prologue, and the first two tiles' out-DMAs split so the DMA bus
    starts draining early.
  - One uniform program (shapes padded to the max class count) serves all
    8 cores -> single compile, single SPMD dispatch.

  Hardware/ISA constraints discovered the hard way (walrus verifier):
  - GPSIMD (Pool) cannot access PSUM at all -> Pool only gets SBUF->SBUF
    work (the sq1 squares); psum evacuation is ACT+DVE only.
  - fp32r matmul operands must be PRODUCED as f32r (dram tensors and
    every producer op declare f32r; a plain f32->f32r bitcast fails
    verification), and fp32r matmuls need even free-dim counts.
  - scalar_tensor_tensor with op1=divide fails the DVE ISA check.
  - ACT Rsqrt is banned in bass (accuracy); sqrt+reciprocal instead.
"""

import math
import sys

for _p in ("/opt/trn_rl_repo", "/opt/pypackages"):
    if _p not in sys.path:
        sys.path.append(_p)

from contextlib import ExitStack

import numpy as np

import concourse.bass as bass
import concourse.tile as tile
from concourse import bacc, mybir
from concourse.bass_utils import run_bass_kernel_spmd

F32 = mybir.dt.float32
F32R = mybir.dt.float32r
BF16 = mybir.dt.bfloat16
AF = mybir.ActivationFunctionType
ALU = mybir.AluOpType

N1, N2 = 8192, 8192
NCORES = 8
DH, DF, NCLS = 64, 32, 8
BANK = 512  # psum bank, in f32 elements

USE_DIVIDE = False  # stt(add, divide) fails the DVE ISA check on TRN2

_cache: dict = {}
_last_nc = None


def _chunks(n, step=BANK):
    return [(c, min(c + step, n)) for c in range(0, n, step)]


def _pipe_chunks(n):
    """Pipeline chunks: two 256-col warmup chunks (fast first-output), then
    512s, with any sub-512 remainder last.  Every chunk fits one psum bank
    and never crosses the 512 grid."""
    out = [(0, 256), (256, 512)]
    c = 512
    while c < n:
        out.append((c, min(c + 512, n)))
        c += 512
    return out


def _build_program(m_pad: int, n_pad: int, m_top: int):
    nc = bacc.Bacc("TRN2", target_bir_lowering=False, debug=False)

    e12_d = nc.dram_tensor("e12", [3, DH + m_pad + n_pad], F32R,
                           kind="ExternalInput").ap()
    w2_d = nc.dram_tensor("w2r", [DH, DF], F32R, kind="ExternalInput").ap()
    wb_d = nc.dram_tensor("wb", [DH, 2], F32, kind="ExternalInput").ap()
    out_d = nc.dram_tensor("out", [m_pad, n_pad], BF16,
                           kind="ExternalOutput").ap()

    n_mt = m_pad // 128
    ch1 = _chunks(m_pad)   # prologue pipeline chunks
    ch2 = _chunks(n_pad)
    mch = _chunks(n_pad)   # main-loop psum chunks (512 grid)

    with tile.TileContext(nc) as tc, ExitStack() as ctx:
        consts = ctx.enter_context(tc.tile_pool(name="consts", bufs=1))
        work = ctx.enter_context(tc.tile_pool(name="work", bufs=1))
        obuf = ctx.enter_context(tc.tile_pool(name="obuf", bufs=6))
        psum = ctx.enter_context(tc.tile_pool(name="psum", bufs=7,
                                              space="PSUM"))
        pnorm = ctx.enter_context(tc.tile_pool(name="pnorm", bufs=1,
                                               space="PSUM"))

        def ps_tile():
            return psum.tile([128, BANK], F32, tag="ps", name="ps")

        # --- t=0: inputs + ACT table warmup --------------------------------
        e12 = consts.tile([3, DH + m_pad + n_pad], F32R)
        w2 = consts.tile([DH, DF], F32R)
        wb = consts.tile([DH, 2], F32)
        ones = consts.tile([DF, DF], F32R)
        onesf = consts.tile([DF, DF], F32)
        warm = consts.tile([1, 1], F32)
        nc.sync.dma_start(e12[:], e12_d)
        nc.sync.dma_start(w2[:], w2_d)
        nc.scalar.dma_start(wb[:], wb_d)
        nc.gpsimd.memset(onesf[:], 1.0)
        nc.gpsimd.tensor_copy(ones[:], onesf[:])  # f32r rounding copy
        # a Sqrt as ACT's first op pins the table set that covers every
        # activation used here (sqrt_and_others: Sqrt/Square/Identity/Copy),
        # so the 1.3us table load runs once, hidden under the input DMAs
        nc.scalar.sqrt(warm[:], wb[0:1, 0:1])

        w1 = e12[:, 0:DH]
        e1t = e12[:, DH:DH + m_pad]
        e2t = e12[:, DH + m_pad:]
        b1 = wb[:, 0:1]
        b2 = wb[0:DF, 1:2]


        h1 = work.tile([DH, m_pad], F32R, tag="h1")
        f1 = work.tile([DF, m_pad], F32R, tag="f1")
        sq1 = work.tile([DF, m_pad], F32R, tag="sq1")
        rn1 = work.tile([128, 2 * n_mt], F32, tag="rn1")
        h2 = work.tile([DH, n_pad], F32R, tag="h2")
        rt2 = work.tile([DF, n_pad], F32, tag="rt2")
        u2 = work.tile([DF, n_pad], F32R, tag="u2")

        # --- prologue: both sides pipelined over column chunks ------------
        # side 2 chunk: L1 mm -> relu(DVE) -> L2 mm -> sq2(ACT) -> ns mm
        #   -> sqrt(ACT) -> u2 = (fps+b2)/rt2 (DVE, fused divide)
        # side 1 chunk: L1 mm -> relu(Pool) -> L2 mm -> f1(ACT,+b2); one
        #   chunk later: sq1(DVE) -> tiny partition-major ns1p mms ->
        #   sqrt(ACT) -> recip(DVE), so the PE/DVE FIFOs never head-block
        #   on the side-1 norm chain.
        sq2 = work.tile([DF, n_pad], F32R, tag="sq2")
        # ns1p is long-lived (read until the last rn1 sqrt) -- it must NOT
        # occupy a slot of the rotating pool, or every wrap-around request
        # would stall behind it
        ns1p = pnorm.tile([128, BANK], F32, tag="ns1p", name="ns1p")

        def emit_l1(side, k):
            """L1 matmul + fused bias-relu; relu2 on Pool, relu1 on DVE."""
            ch, et = (ch1, e1t) if side == 1 else (ch2, e2t)
            c0, c1 = ch[k]
            w = c1 - c0
            hp = ps_tile()
            nc.tensor.matmul(hp[0:DH, 0:w], w1,
                             et[:, c0:c1],
                             start=True, stop=True)
            if side == 2:
                nc.scalar.activation(h2[:, c0:c1], hp[0:DH, 0:w], AF.Relu,
                                     bias=b1, scale=1.0)
            else:
                nc.vector.tensor_scalar(h1[:, c0:c1], hp[0:DH, 0:w], b1,
                                        0.0, ALU.add, ALU.max)

        def norm_sq(c0, c1):
            nc.gpsimd.tensor_tensor(sq1[:, c0:c1], f1[:, c0:c1],
                                    f1[:, c0:c1], ALU.mult)

        def norm_fin(c0, c1):
            # fp32r matmuls need even free counts, so each norm matmul
            # writes a 2-wide column pair; rn1 is indexed at stride 2
            lo, hi = (c0 + 127) // 128, c1 // 128
            for m in range(lo, hi):
                nc.tensor.matmul(
                    ns1p[:, 2 * m:2 * m + 2],
                    sq1[:, m * 128:(m + 1) * 128],
                    ones[:, 0:2], start=True, stop=True)
            if hi > lo:
                nc.scalar.sqrt(rn1[:, 2 * lo:2 * hi], ns1p[:, 2 * lo:2 * hi])
                nc.vector.reciprocal(rn1[:, 2 * lo:2 * hi],
                                     rn1[:, 2 * lo:2 * hi])

        # ---- main-loop helpers (needed mid-prologue for early emission) --
        nk = len(mch)
        obs = [None] * n_mt
        pss = {}
        # time-aware copy-engine schedule: Pool finishes its prologue work
        # first, so it front-loads the early copies; ACT frees next, DVE
        # last (u2/sq1 tail)
        big_rot = ["a", "v", "a", "v", "a", "v", "a", "v", "a", "a",
                   "v", "a", "v", "a", "v", "a", "v", "a"]
        small_rot = ["v", "a", "v", "v", "a", "v", "v", "a", "v"]
        nbig = [0]
        nsmall = [0]

        def emit_mm(m, k):
            c0, c1 = mch[k]
            w = c1 - c0
            ps = ps_tile()
            pss[(m, k)] = ps
            nc.tensor.matmul(ps[:, 0:w],
                             f1[:, m * 128:(m + 1) * 128],
                             u2[:, c0:c1],
                             start=True, stop=True)

        def emit_copy(m, k):
            c0, c1 = mch[k]
            w = c1 - c0
            ps = pss.pop((m, k))
            ob = obs[m]
            scale = rn1[:, 2 * m:2 * m + 1]
            if w >= 256:
                r = big_rot[nbig[0] % len(big_rot)]
                nbig[0] += 1
            else:
                r = small_rot[nsmall[0] % len(small_rot)]
                nsmall[0] += 1
            if r == "a":
                nc.scalar.activation(ob[:, c0:c1], ps[:, 0:w], AF.Copy,
                                     bias=0.0, scale=scale)
            else:
                nc.vector.tensor_scalar(ob[:, c0:c1], ps[:, 0:w], scale,
                                        None, ALU.mult)

        # Choreographed emission: every engine's in-order queue receives its
        # ops in expected-readiness order.  L1 matmuls + relus for chunk k+1
        # are pre-emitted; side-2's norm chain leads; f1 (Pool) and the
        # side-1 norm pieces ride the gaps; the first column-0 main
        # matmuls are emitted mid-prologue so the PE reaches them the moment
        # u2(0) lands.
        n_ch = max(len(ch1), len(ch2))
        emit_l1(2, 0)
        emit_l1(1, 0)
        if len(ch2) > 1:
            emit_l1(2, 1)
        if len(ch1) > 1:
            emit_l1(1, 1)
        early_mains = []
        done_norms = set()
        for k in range(n_ch):
            in1 = k < len(ch1)
            in2 = k < len(ch2)
            if k + 2 < len(ch2):
                emit_l1(2, k + 2)
            if k + 2 < len(ch1):
                emit_l1(1, k + 2)
            if k == 1:
                norm_sq(128, ch1[0][1])
            if in2:
                c0, c1 = ch2[k]
                w = c1 - c0
                fp2 = ps_tile()
                nc.tensor.matmul(fp2[0:DF, 0:w], w2,
                                 h2[:, c0:c1],
                                 start=True, stop=True)
                nc.scalar.activation(sq2[:, c0:c1], fp2[0:DF, 0:w],
                                     AF.Square, bias=b2, scale=1.0)
            if in1:
                d0, d1 = ch1[k]
                wd = d1 - d0
                fp1 = ps_tile()
                nc.tensor.matmul(fp1[0:DF, 0:wd], w2,
                                 h1[:, d0:d1],
                                 start=True, stop=True)
                nc.scalar.activation(f1[:, d0:d1], fp1[0:DF, 0:wd],
                                     AF.Identity, bias=b2, scale=1.0)
            if k == 0:
                # m-tile-0 fast path: sq1 slots into DVE's idle window just
                # before u2(0), so rn1[0] lands right after the first mains
                norm_sq(0, 128)

            if in2:
                np_ = ps_tile()
                nc.tensor.matmul(np_[0:DF, 0:w], ones,
                                 sq2[:, c0:c1],
                                 start=True, stop=True)
                nc.scalar.sqrt(rt2[:, c0:c1], np_[0:DF, 0:w])
                if USE_DIVIDE:
                    nc.vector.scalar_tensor_tensor(
                        u2[:, c0:c1], fp2[0:DF, 0:w], b2, rt2[:, c0:c1],
                        ALU.add, ALU.divide)
                else:
                    nc.vector.reciprocal(rt2[:, c0:c1], rt2[:, c0:c1])
                    nc.vector.scalar_tensor_tensor(
                        u2[:, c0:c1], fp2[0:DF, 0:w], b2, rt2[:, c0:c1],
                        ALU.add, ALU.mult)
            if k == 0:
                norm_fin(0, 128)
                done_norms.add(0)
            if k == 1:
                norm_fin(128, ch1[0][1])
                for m in range(2):
                    obs[m] = obuf.tile([128, n_pad], BF16, tag="ob",
                                       name="ob")
                    emit_mm(m, 0)
                early_mains.append(2)
        # side-1 norm chunks 1+ are deferred into the mains stream (their
        # rn1 slices are only read by later m-tiles' copies)

        # --- main loop ----------------------------------------------------
        # steady state: finish tile m, then start tile m+LEAD's column 0.
        LEAD = 5
        n_early = early_mains[0] if early_mains else 0
        for m in range(n_early, min(LEAD, n_mt)):
            obs[m] = obuf.tile([128, n_pad], BF16, tag="ob", name="ob")
            emit_mm(m, 0)
        c0end = mch[0][1]
        for m in range(n_mt):
            emit_copy(m, 0)
            if m < 2:
                # early tiles: ship column-chunk 0 the moment it is copied,
                # so the DMA bus starts draining sooner; last tiles: halve
                # the final transfer so the bus tail is shorter
                r1c = min((m + 1) * 128, m_top)
                if r1c > m * 128:
                    nc.sync.dma_start(out_d[m * 128:r1c, 0:c0end],
                                      obs[m][0:r1c - m * 128, 0:c0end])
            for k in range(1, nk):
                emit_mm(m, k)
                emit_copy(m, k)
            if m < len(ch1) - 1:
                # deferred side-1 norm pieces ride between m-tiles
                norm_sq(ch1[m + 1][0], ch1[m + 1][1])
                norm_fin(ch1[m + 1][0], ch1[m + 1][1])
            if m + LEAD < n_mt:
                obs[m + LEAD] = obuf.tile([128, n_pad], BF16, tag="ob",
                                          name="ob")
                emit_mm(m + LEAD, 0)
            r0 = m * 128
            r1 = min((m + 1) * 128, m_top)
            if r1 > r0:
                if m < 2:
                    nc.sync.dma_start(out_d[r0:r1, c0end:], obs[m][0:r1 - r0,
                                                                  c0end:])
                else:
                    nc.sync.dma_start(out_d[r0:r1, :], obs[m][0:r1 - r0, :])

    nc.compile()
    return nc


def kernel(**inputs) -> np.ndarray:
    global _last_nc
    edges1 = np.ascontiguousarray(np.asarray(inputs["edges1"], dtype=np.float32))
    edges2 = np.ascontiguousarray(np.asarray(inputs["edges2"], dtype=np.float32))
    W1 = np.asarray(inputs["W1"], dtype=np.float32)
    b1 = np.asarray(inputs["b1"], dtype=np.float32)
    W2 = np.asarray(inputs["W2"], dtype=np.float32)
    b2 = np.asarray(inputs["b2"], dtype=np.float32)

    cls1 = edges1[:, 3].astype(np.int64)
    cls2 = edges2[:, 3].astype(np.int64)
    order1 = np.argsort(cls1, kind="stable")
    order2 = np.argsort(cls2, kind="stable")
    cnt1 = np.bincount(cls1, minlength=NCLS)
    cnt2 = np.bincount(cls2, minlength=NCLS)
    b1_ = np.concatenate([[0], np.cumsum(cnt1)]).astype(int)
    b2_ = np.concatenate([[0], np.cumsum(cnt2)]).astype(int)

    m_pad = max(128, math.ceil(cnt1.max() / 128) * 128)
    n_pad = max(8, math.ceil(cnt2.max() / 8) * 8)
    m_top = max(8, math.ceil(cnt1.max() / 8) * 8)  # valid-row DMA bound

    key = (m_pad, n_pad, m_top)
    if key not in _cache:
        _cache[key] = _build_program(m_pad, n_pad, m_top)
    nc = _cache[key]
    _last_nc = nc

    # wb: [64, 2] = b1 | b2
    wb = np.zeros((DH, 2), dtype=np.float32)
    wb[:, 0] = b1
    wb[0:DF, 1] = b2

    in_maps = []
    for c in range(NCORES):
        rows = order1[b1_[c]:b1_[c + 1]]
        cols = order2[b2_[c]:b2_[c + 1]]
        e12 = np.zeros((3, DH + m_pad + n_pad), dtype=np.float32)
        e12[:, 0:DH] = W1
        e12[:, DH:DH + len(rows)] = edges1[rows, :3].T
        e12[:, DH + m_pad:DH + m_pad + len(cols)] = edges2[cols, :3].T
        in_maps.append({"e12": e12, "w2r": W2, "wb": wb})

    res = run_bass_kernel_spmd(nc, in_maps, core_ids=list(range(NCORES)))

    out = np.zeros((N1, N2), dtype=np.float32)
    for c in range(NCORES):
        rows = order1[b1_[c]:b1_[c + 1]]
        cols = order2[b2_[c]:b2_[c + 1]]
        blk = np.asarray(res.results[c]["out"])[:len(rows), :len(cols)]
        out[np.ix_(rows, cols)] = blk.astype(np.float32)
    return out
